# revision 1
# baseline (speedup 1.0000x reference)
"""Cosine-similarity kernel for trn2: out = l2norm_rows(x) @ l2norm_rows(W).

x: [65536, 512] f32, W: [512, 462] f32 -> out: [65536, 462] f32.

Strategy (data-parallel over 8 cores, batch-sharded x, replicated W):
  The host hands each core x^T for its batch shard (layout marshaling only)
  so the contraction dim (in_dim) lands on SBUF partitions with perfect
  DMA descriptors and ZERO on-chip transposes.

  Per core (8192 batch columns):
  - Normalize W on-chip once (rows of W over the free axis), stored f32r.
  - GEMM produces out^T: stationary = wn chunk [128i, ~116o], moving =
    x^T chunk [128i, 512b], f32r (1 cycle/row), accumulated over 4 K-chunks.
  - Row sumsq: ACT squares x^T chunks (f32r), then a ones[128,1]-stationary
    matmul reduces over partitions -> ssq [1, 512b] in PSUM.
  - rsqrt(max(ssq, eps)) on one lane; broadcast across partitions with a
    K=1 matmul (ones[1,128] stationary, s[1,512] moving) -> s_bc [128, 512].
  - DVE multiplies each out^T PSUM chunk by s_bc (fused scale + copy).
  - out^T DMA'd back; host transposes to natural layout.
"""

from contextlib import ExitStack

import numpy as np

import concourse.bass as bass
import concourse.mybir as mybir
import concourse.tile as tile
from concourse import bacc, bass_utils
from concourse.bass import ds

N_CORES = 8
B = 65536
B_PER = B // N_CORES          # 8192 batch columns per core
IN_DIM = 512
OUT_DIM = 462
EPS = 1e-12
P = 128
KC = IN_DIM // P              # 4 contraction chunks
NB = 512                      # batch columns per matmul (moving-dim max f32)
GROUP_COLS = 1024             # batch columns per DMA group (2 MB in)
SUBS = GROUP_COLS // NB       # sub-tiles per group
N_GROUPS = B_PER // GROUP_COLS
O_CHUNKS = [(0, 128), (128, 128), (256, 128), (384, 78)]  # 462 = 3*128+78

F32 = mybir.dt.float32
F32R = mybir.dt.float32r


def _build_bass():
    nc = bacc.Bacc("TRN2", debug=False, num_devices=N_CORES)
    xt_d = nc.dram_tensor("xt", [IN_DIM, B_PER], F32R, kind="ExternalInput").ap()
    w_d = nc.dram_tensor("w", [IN_DIM, OUT_DIM], F32, kind="ExternalInput").ap()
    ot_d = nc.dram_tensor("ot", [OUT_DIM, B_PER], F32, kind="ExternalOutput").ap()

    with ExitStack() as ctx:
        tc = ctx.enter_context(tile.TileContext(nc))

        singles = ctx.enter_context(tc.tile_pool(name="singles", bufs=1))
        xpool = ctx.enter_context(tc.tile_pool(name="xin", bufs=2))
        sqpool = ctx.enter_context(tc.tile_pool(name="sq", bufs=2))
        opool = ctx.enter_context(tc.tile_pool(name="oout", bufs=2))
        stats = ctx.enter_context(tc.tile_pool(name="stats", bufs=4))
        psum_o = ctx.enter_context(tc.tile_pool(name="psum_o", bufs=4, space="PSUM"))
        psum_s = ctx.enter_context(tc.tile_pool(name="psum_s", bufs=2, space="PSUM"))
        psum_b = ctx.enter_context(tc.tile_pool(name="psum_b", bufs=2, space="PSUM"))

        zero_bias = singles.tile([P, 1], F32)
        nc.vector.memset(zero_bias, 0.0)
        ones_f = singles.tile([P, 1], F32)
        nc.vector.memset(ones_f, 1.0)
        ones_k = singles.tile([P, 1], F32R)   # reduce-over-partitions stationary
        nc.vector.tensor_copy(out=ones_k, in_=ones_f)
        ones_mf = singles.tile([1, P], F32)
        nc.vector.memset(ones_mf, 1.0)
        ones_m = singles.tile([1, P], F32R)   # K=1 broadcast stationary
        nc.vector.tensor_copy(out=ones_m, in_=ones_mf)

        # ---- W normalization (once) ----
        w_sb = singles.tile([P, KC, OUT_DIM], F32)
        nc.sync.dma_start(w_sb, w_d.rearrange("(c p) o -> p c o", p=P))
        wsq = singles.tile([P, KC, OUT_DIM], F32)  # scratch squares
        wssq = singles.tile([P, KC], F32)
        for c in range(KC):
            nc.scalar.activation(
                out=wsq[:, c, :],
                in_=w_sb[:, c, :],
                func=mybir.ActivationFunctionType.Square,
                bias=zero_bias,
                accum_out=wssq[:, c : c + 1],
            )
        nc.vector.tensor_scalar_max(wssq, wssq, EPS)
        nc.scalar.activation(
            out=wssq, in_=wssq, func=mybir.ActivationFunctionType.Sqrt, bias=zero_bias
        )
        wrs = singles.tile([P, KC], F32)
        nc.vector.reciprocal(wrs, wssq)
        # f32r so the PE matmul runs at 1 cycle/row; producer rounds to f32r
        wn_sb = singles.tile([P, KC, OUT_DIM], F32R)
        for c in range(KC):
            nc.vector.tensor_scalar_mul(wn_sb[:, c, :], w_sb[:, c, :], wrs[:, c : c + 1])

        # ---- main loop ----
        xt_v = xt_d.rearrange("(c p) b -> p c b", p=P)  # [128, KC, B_PER]
        # out^T DRAM view by o-chunk: row = oc*116 + p
        for g in range(N_GROUPS):
            b0 = g * GROUP_COLS
            x_sb = xpool.tile([P, KC, GROUP_COLS], F32R)
            nc.sync.dma_start(x_sb, xt_v[:, :, ds(b0, GROUP_COLS)])

            ot_sb = opool.tile([P, len(O_CHUNKS), GROUP_COLS], F32)

            for s in range(SUBS):
                sl = ds(s * NB, NB)

                # --- row sumsq: squares on ACT, partition-reduce on PE ---
                xsq = sqpool.tile([P, KC, NB], F32R)
                for c in range(KC):
                    nc.scalar.activation(
                        out=xsq[:, c, :],
                        in_=x_sb[:, c, sl],
                        func=mybir.ActivationFunctionType.Square,
                        bias=zero_bias,
                    )
                ps_ssq = psum_s.tile([1, NB], F32)
                for c in range(KC):
                    nc.tensor.matmul(
                        ps_ssq,
                        lhsT=ones_k[:, :],
                        rhs=xsq[:, c, :],
                        start=(c == 0),
                        stop=(c == KC - 1),
                    )
                # eps-max on one lane (fast), then broadcast ssq across
                # partitions via K=1 matmul so sqrt+reciprocal run 128-wide
                # (a [1,N] reciprocal is ~3us of single-lane DVE time).
                ssq_r = stats.tile([1, NB], F32R)
                with nc.allow_low_precision(reason="f32r for PE broadcast"):
                    nc.vector.tensor_scalar_max(ssq_r, ps_ssq, EPS)
                ps_bc = psum_b.tile([P, NB], F32)
                nc.tensor.matmul(ps_bc, lhsT=ones_m[:, :], rhs=ssq_r[:, :])
                sq_bc = stats.tile([P, NB], F32)
                nc.scalar.activation(
                    out=sq_bc,
                    in_=ps_bc,
                    func=mybir.ActivationFunctionType.Sqrt,
                    bias=zero_bias,
                )
                s_bc = stats.tile([P, NB], F32)
                nc.vector.reciprocal(s_bc, sq_bc)

                # --- GEMM: out^T chunks = wn_chunk.T @ x^T chunk ---
                for oc, (o0, osz) in enumerate(O_CHUNKS):
                    po = psum_o.tile([P, NB], F32)
                    for c in range(KC):
                        nc.tensor.matmul(
                            po[:osz, :],
                            lhsT=wn_sb[:, c, ds(o0, osz)],
                            rhs=x_sb[:, c, sl],
                            start=(c == 0),
                            stop=(c == KC - 1),
                        )
                    # fused scale-by-rsqrt + PSUM->SBUF copy
                    nc.vector.tensor_mul(
                        ot_sb[:osz, oc, ds(s * NB, NB)],
                        po[:osz, :],
                        s_bc[:osz, :],
                    )

            # out^T group store: DRAM row = oc*128 + p. The first three chunks
            # are 128 rows; the last is 78, so it gets its own DMA.
            dst_a = bass.AP(
                tensor=ot_d.tensor,
                offset=b0,
                ap=[[B_PER, 128], [128 * B_PER, 3], [1, GROUP_COLS]],
            )
            nc.scalar.dma_start(dst_a, ot_sb[:128, 0:3, :])
            dst_b = bass.AP(
                tensor=ot_d.tensor,
                offset=384 * B_PER + b0,
                ap=[[B_PER, 78], [1, GROUP_COLS]],
            )
            nc.scalar.dma_start(dst_b, ot_sb[:78, 3, :])

    nc.compile()
    return nc


_NC_CACHE = None
LAST_RESULTS = None  # BassKernelResults of the most recent run (for profiling)


def kernel(x: np.ndarray, W: np.ndarray) -> np.ndarray:
    global _NC_CACHE, LAST_RESULTS
    if _NC_CACHE is None:
        _NC_CACHE = _build_bass()
    nc = _NC_CACHE

    x = np.asarray(x, dtype=np.float32)
    W = np.ascontiguousarray(np.asarray(W, dtype=np.float32))
    in_maps = []
    for i in range(N_CORES):
        shard = np.ascontiguousarray(x[i * B_PER : (i + 1) * B_PER].T)
        in_maps.append({"xt": shard, "w": W})
    res = bass_utils.run_bass_kernel_spmd(nc, in_maps, core_ids=list(range(N_CORES)))
    LAST_RESULTS = res
    out = np.concatenate(
        [np.ascontiguousarray(r["ot"].T) for r in res.results], axis=0
    )
    return out

